# revision 5
# baseline (speedup 1.0000x reference)
"""Trainium2 Bass kernel for hash-indexed per-pixel conv (nn_ABC_2D).

Reference computation:
    patches[b,p,c] = x.reshape(B,-1)[b, hash_idx[p,c]]        # [B,P,CK] gather
    out[b,k,p]     = sum_c weights[p,k,c] * patches[b,p,c]   # per-pixel matmul

Sharding: pixels P=16384 split across 8 cores (2048 each); each core streams
its weight slab and patch slab and computes all B=8 batches.

Device kernel structure (per core, per 256-pixel tile):
  - weights arrive pre-transposed [c, p, k] and patches [c, p, b] (c on
    partitions), so TensorE contracts over c directly: stationary =
    w[c,(8p,16k)] (128 cols), moving = patches[c,(8p,8b)] (64 cols), PSUM
    [(p,k), (p',b)]; 2 matmuls per 8-pixel group (c split 128+16).
  - DMA layout is engineered around the 16-SDMA-engine port map (each
    engine owns 8 fixed partitions; per-engine ~27GB/s caps the stream):
    the main chunk (c<128) of weights AND patches for a tile travels as ONE
    [128, 6144] DMA from an interleaved DRAM tensor -- balanced across all
    16 engines, few large descriptors.  The 16-channel tails (w+patch
    merged, [16, 6144]) park in per-iteration holders at partition base
    32*(t%3) (PE quadrant 3 unusable), rotating so the reachable engine
    quads take turns; tail matmuls use the PE tile_position row-group
    mechanism (stationary rows at base q, rhs partitions to match).
  - the diagonal blocks (p'==p) are extracted with a mask multiply +
    tensor_reduce over p' (engine APs cannot start at 16-aligned
    partitions, so no sub-32 partition slicing is possible).
  - weights/patches travel in bf16 (host converts; ~halves HBM traffic;
    rel err ~2.4e-3 vs the 2e-2 gate).  PSUM/output f32.

The hash gather itself is done on the host: every device-side fine-grained
gather path was measured or ruled out on silicon (indirect_dma_start
consumes one index per partition per instruction; dma_gather needs 256-byte
elements; GpSimd ap_gather tables cap at 128KB per 16-partition group).

Output is returned per-core as [128, 2048] = [(p%8,k), (tile, grp, b)] and
reassembled/permuted on the host.
"""
import numpy as np

B, C, H, W = 8, 16, 128, 128
P = H * W            # 16384
KN = 16
CK = C * 9           # 144
NCORES = 8
PPC = P // NCORES    # 2048 pixels per core
C0 = 128
C1 = CK - C0         # 16
GRP = 8              # pixels per matmul group (8px*16k = 128 stationary cols)
TILE_P = 256         # pixels per tile
GRPS_PER_TILE = TILE_P // GRP      # 32
NTILES = PPC // TILE_P             # 8
WCOL = TILE_P * KN                 # 4096 weight cols per tile
GCOL = TILE_P * B                  # 2048 patch cols per tile
MCOL = WCOL + GCOL                 # 6144 merged cols per tile

_CACHE = {}


def build(reps=1):
    from concourse import bacc, bass, mybir, tile

    wire_dt = mybir.dt.bfloat16
    nc = bacc.Bacc(None)
    ppc = NTILES * TILE_P
    # Main chunk (c<128): per tile, weight cols then patch cols, interleaved
    # tile-major so each tile is one contiguous [128, 6144] DMA.
    m0 = nc.declare_dram_parameter("m0", [C0, NTILES * MCOL], wire_dt,
                                   isOutput=False)
    # Tail (c in 128..144), same merged layout.
    m1 = nc.declare_dram_parameter("m1", [C1, NTILES * MCOL], wire_dt,
                                   isOutput=False)
    msk = nc.declare_dram_parameter("msk", [128, GRPS_PER_TILE * GRP * B],
                                    mybir.dt.float32, isOutput=False)
    out = nc.declare_dram_parameter("out", [128, ppc], mybir.dt.float32, isOutput=True)

    with tile.TileContext(nc) as tc:
        with (
            tc.tile_pool(name="wp", bufs=3) as wp,
            tc.tile_pool(name="th", bufs=2) as th,
            tc.tile_pool(name="sp", bufs=3) as sp,
            tc.tile_pool(name="op", bufs=1) as op,
            tc.tile_pool(name="ps", bufs=2, space="PSUM") as ps,
        ):
            o_sb = op.tile([128, ppc], mybir.dt.float32)
            msk_sb = op.tile([128, GRPS_PER_TILE * GRP * B], mybir.dt.float32)
            nc.sync.dma_start(out=msk_sb[:], in_=msk[:])

            def body(_iv=None):
                # Per-iteration holders for the 16-channel tails: tile t at
                # partition base 32*(t%3), column slot t//3.
                t1h = th.tile([128, 3 * MCOL], wire_dt, tag="t1h")
                for t in range(NTILES):
                    q = 32 * (t % 3)
                    h = t // 3
                    mt = wp.tile([C0, MCOL], wire_dt, tag="m0")
                    nc.sync.dma_start(
                        out=mt[:], in_=m0[:, t * MCOL:(t + 1) * MCOL])
                    nc.scalar.dma_start(
                        out=t1h[q:q + C1, h * MCOL:(h + 1) * MCOL],
                        in_=m1[:, t * MCOL:(t + 1) * MCOL])

                    ps_t = ps.tile([128, GRPS_PER_TILE * GRP * B], mybir.dt.float32,
                                   space="PSUM", tag="acc")
                    for g in range(GRPS_PER_TILE):
                        pix = g * GRP
                        nc.tensor.matmul(
                            out=ps_t[:, g * GRP * B:(g + 1) * GRP * B],
                            lhsT=mt[:, pix * KN:(pix + GRP) * KN],
                            rhs=mt[:, WCOL + pix * B:
                                   WCOL + (pix + GRP) * B].rearrange(
                                       "c (p b) -> c p b", p=GRP),
                            start=True, stop=False)
                        nc.tensor.matmul(
                            out=ps_t[:, g * GRP * B:(g + 1) * GRP * B],
                            lhsT=t1h[q:q + C1,
                                     h * MCOL + pix * KN:
                                     h * MCOL + (pix + GRP) * KN],
                            rhs=t1h[q:q + C1,
                                    h * MCOL + WCOL + pix * B:
                                    h * MCOL + WCOL + (pix + GRP) * B].rearrange(
                                        "c (p b) -> c p b", p=GRP),
                            start=False, stop=True)
                    # Diagonal extraction without sub-32 partition slicing:
                    # mask out off-diagonal pixel columns, then reduce over p'.
                    s_t = sp.tile([128, GRPS_PER_TILE * GRP * B], mybir.dt.float32,
                                  tag="st")
                    nc.vector.tensor_tensor(
                        out=s_t[:], in0=ps_t[:], in1=msk_sb[:],
                        op=mybir.AluOpType.mult)
                    nc.vector.tensor_reduce(
                        out=o_sb[:, t * TILE_P:(t + 1) * TILE_P].rearrange(
                            "q (G b) -> q G b", G=GRPS_PER_TILE, b=B),
                        in_=s_t[:].rearrange(
                            "q (G p b) -> q G b p", G=GRPS_PER_TILE, p=GRP, b=B),
                        axis=mybir.AxisListType.X,
                        op=mybir.AluOpType.add)

            if reps == 1:
                body()
            else:
                with tc.For_i(0, reps, 1) as _i:
                    body(_i)
            nc.sync.dma_start(out=out[:], in_=o_sb[:])
    nc.finalize()
    return nc


def make_mask():
    """mask[(pl,k), (G,p',b)] = 1 when p' == pl."""
    m = np.zeros((GRP, KN, GRPS_PER_TILE, GRP, B), dtype=np.float32)
    for pl in range(GRP):
        m[pl, :, :, pl, :] = 1.0
    return m.reshape(128, GRPS_PER_TILE * GRP * B)


def prep_host_inputs(x, weights, hash_idx):
    """Per-core input maps (patches gathered in numpy, all wires bf16).

    m0/m1 layout: [c, (tile, wcols+gcols)] where per tile the first 4096
    cols are weights [p,k] and the next 2048 are patches [p,b]."""
    import ml_dtypes
    B_, Cc, Hh, Ww = x.shape
    flat = np.asarray(x).reshape(B_, -1)
    msk = make_mask()
    in_maps = []
    for c in range(NCORES):
        sl = slice(c * PPC, (c + 1) * PPC)
        w_t = np.asarray(weights)[sl].transpose(2, 0, 1)      # [CK,PPC,KN]
        idx_t = np.asarray(hash_idx)[sl].T                    # [CK,PPC]
        pat = flat[:, idx_t].transpose(1, 2, 0)               # [CK,PPC,B]
        w_t = w_t.reshape(CK, NTILES, WCOL)
        pat = pat.reshape(CK, NTILES, GCOL)
        m = np.concatenate([w_t, pat], axis=2).astype(ml_dtypes.bfloat16)
        m = m.reshape(CK, NTILES * MCOL)
        in_maps.append({
            "m0": np.ascontiguousarray(m[:C0]),
            "m1": np.ascontiguousarray(m[C0:]),
            "msk": msk,
        })
    return in_maps


def assemble(results, ppc=PPC):
    """Per-core o[(pl,k), (t,G,b)] -> full [B, KN, P]; p = t*256 + G*8 + pl."""
    outs = []
    for r in results:
        o = r["out"].reshape(GRP, KN, ppc // TILE_P, GRPS_PER_TILE, B)
        o = o.transpose(4, 1, 2, 3, 0).reshape(B, KN, ppc)
        outs.append(o)
    return np.concatenate(outs, axis=2)


def kernel(x, weights, hash_idx):
    from concourse.bass_utils import run_bass_kernel_spmd

    if "nc" not in _CACHE:
        _CACHE["nc"] = build()
    nc = _CACHE["nc"]
    in_maps = prep_host_inputs(np.asarray(x), np.asarray(weights),
                               np.asarray(hash_idx))
    res = run_bass_kernel_spmd(nc, in_maps, list(range(NCORES)))
    return assemble(res.results)


# revision 19
# speedup vs baseline: 1.8388x; 1.8388x over previous
"""Trainium2 Bass kernel for hash-indexed per-pixel conv (nn_ABC_2D).

Reference computation:
    patches[b,p,c] = x.reshape(B,-1)[b, hash_idx[p,c]]        # [B,P,CK] gather
    out[b,k,p]     = sum_c weights[p,k,c] * patches[b,p,c]   # per-pixel matmul

Sharding: pixels P=16384 split across 8 cores (2048 each); each core streams
its weight slab and patch slab and computes all B=8 batches.

Device kernel structure (per core, per 256-pixel tile):
  - weights arrive pre-transposed [c, p, k] and patches [c, p, b] (c on
    partitions), so TensorE contracts over c directly: stationary =
    w[c,(8p,16k)] (128 cols), moving = patches[c,(8p,8b)] (64 cols), PSUM
    [(p,k), (p',b)]; 2 matmuls per 8-pixel group (c split 128+16).
  - DMA layout is engineered around the 16-SDMA-engine port map (each
    engine owns 8 fixed partitions; per-engine ~27GB/s caps the stream):
    the main chunk (c<128) of weights AND patches for a tile travels as ONE
    [128, 6144] DMA from an interleaved DRAM tensor -- balanced across all
    16 engines, few large descriptors.  The 16-channel tails (w+patch
    merged, [16, 6144]) park in per-iteration holders at partition base
    32*(t%3) (PE quadrant 3 unusable), rotating so the reachable engine
    quads take turns; tail matmuls use the PE tile_position row-group
    mechanism (stationary rows at base q, rhs partitions to match).
  - the diagonal blocks (p'==p) are extracted with a mask multiply +
    tensor_reduce over p' (engine APs cannot start at 16-aligned
    partitions, so no sub-32 partition slicing is possible).
  - weights/patches travel in bf16 (host converts; ~halves HBM traffic;
    rel err ~2.4e-3 vs the 2e-2 gate).  PSUM/output f32.

The hash gather itself is done on the host: every device-side fine-grained
gather path was measured or ruled out on silicon (indirect_dma_start
consumes one index per partition per instruction; dma_gather needs 256-byte
elements; GpSimd ap_gather tables cap at 128KB per 16-partition group).

Output is returned per-core as [128, 2048] = [(p%8,k), (tile, grp, b)] and
reassembled/permuted on the host.
"""
import numpy as np

B, C, H, W = 8, 16, 128, 128
P = H * W            # 16384
KN = 16
CK = C * 9           # 144
NCORES = 8
PPC = P // NCORES    # 2048 pixels per core
C0 = 128
C1 = CK - C0         # 16
GRP = 8              # pixels per matmul group (8px*16k = 128 stationary cols)
TILE_P = 256         # pixels per tile
GRPS_PER_TILE = TILE_P // GRP      # 32
NTILES = PPC // TILE_P             # 8
WCOL = TILE_P * KN                 # 4096 weight cols per tile
GCOL = TILE_P * B                  # 2048 patch cols per tile
MCOL = WCOL + GCOL                 # 6144 merged cols per tile

_CACHE = {}


EXTRACT = "dve"     # "dve" or "act" (ScalarE copies PSUM->SBUF bf16 first)


def build(reps=1, unroll=False, extract=None):
    from concourse import bacc, bass, mybir, tile

    extract = EXTRACT if extract is None else extract
    wire_dt = mybir.dt.bfloat16
    nc = bacc.Bacc(None)
    ppc = NTILES * TILE_P
    # Main chunk (c<128): per tile, weight cols then patch cols, interleaved
    # tile-major so each tile is one contiguous [128, 6144] DMA.
    m0 = nc.declare_dram_parameter("m0", [C0, NTILES * MCOL], wire_dt,
                                   isOutput=False)
    # Tail (c in 128..144), same merged layout.
    m1 = nc.declare_dram_parameter("m1", [C1, NTILES * MCOL], wire_dt,
                                   isOutput=False)
    msk_dt = mybir.dt.bfloat16 if extract == "act" else mybir.dt.float32
    msk = nc.declare_dram_parameter("msk", [128, GRPS_PER_TILE * GRP * B],
                                    msk_dt, isOutput=False)
    if extract == "mix":
        mskb = nc.declare_dram_parameter("mskb", [128, GRPS_PER_TILE * GRP * B],
                                         mybir.dt.bfloat16, isOutput=False)
    out = nc.declare_dram_parameter("out", [128, ppc], mybir.dt.float32, isOutput=True)

    with tile.TileContext(nc) as tc:
        with (
            tc.tile_pool(name="wp", bufs=4) as wp,
            tc.tile_pool(name="th", bufs=2) as th,
            tc.tile_pool(name="sp", bufs=3) as sp,
            tc.tile_pool(name="op", bufs=1) as op,
            tc.tile_pool(name="ps", bufs=2, space="PSUM") as ps,
        ):
            o_sb = op.tile([128, ppc], mybir.dt.float32)
            msk_sb = op.tile([128, GRPS_PER_TILE * GRP * B], msk_dt)
            nc.sync.dma_start(out=msk_sb[:], in_=msk[:])
            if extract == "mix":
                mskb_sb = op.tile([128, GRPS_PER_TILE * GRP * B],
                                  mybir.dt.bfloat16)
                nc.sync.dma_start(out=mskb_sb[:], in_=mskb[:])

            def body(_iv=None):
                # Per-iteration holders for the 16-channel tails: tile t at
                # partition base 32*(t%3), column slot t//3.
                t1h = th.tile([128, 3 * MCOL], wire_dt, tag="t1h")
                for t in range(NTILES):
                    q = 32 * (t % 3)
                    h = t // 3
                    mt = wp.tile([C0, MCOL], wire_dt, tag="m0")
                    nc.sync.dma_start(
                        out=mt[:], in_=m0[:, t * MCOL:(t + 1) * MCOL])
                    nc.sync.dma_start(
                        out=t1h[q:q + C1, h * MCOL:(h + 1) * MCOL],
                        in_=m1[:, t * MCOL:(t + 1) * MCOL])

                    ps_t = ps.tile([128, GRPS_PER_TILE * GRP * B], mybir.dt.float32,
                                   space="PSUM", tag="acc")
                    for g in range(GRPS_PER_TILE):
                        pix = g * GRP
                        nc.tensor.matmul(
                            out=ps_t[:, g * GRP * B:(g + 1) * GRP * B],
                            lhsT=mt[:, pix * KN:(pix + GRP) * KN],
                            rhs=mt[:, WCOL + pix * B:
                                   WCOL + (pix + GRP) * B].rearrange(
                                       "c (p b) -> c p b", p=GRP),
                            start=True, stop=False)
                        nc.tensor.matmul(
                            out=ps_t[:, g * GRP * B:(g + 1) * GRP * B],
                            lhsT=t1h[q:q + C1,
                                     h * MCOL + pix * KN:
                                     h * MCOL + (pix + GRP) * KN],
                            rhs=t1h[q:q + C1,
                                    h * MCOL + WCOL + pix * B:
                                    h * MCOL + WCOL + (pix + GRP) * B].rearrange(
                                        "c (p b) -> c p b", p=GRP),
                            start=False, stop=True)
                    # Diagonal extraction without sub-32 partition slicing:
                    # mask out off-diagonal pixel columns, then reduce over p'.
                    # "mix": odd tiles go ScalarE(copy) -> GpSimd(mult+reduce)
                    # so DVE only handles half the tiles.
                    o_slc = o_sb[:, t * TILE_P:(t + 1) * TILE_P].rearrange(
                        "q (G b) -> q G b", G=GRPS_PER_TILE, b=B)
                    if extract == "mix" and t % 2 == 1:
                        c_t = sp.tile([128, GRPS_PER_TILE * GRP * B],
                                      mybir.dt.bfloat16, tag="ct")
                        nc.scalar.copy(out=c_t[:], in_=ps_t[:])
                        s_t = sp.tile([128, GRPS_PER_TILE * GRP * B],
                                      mybir.dt.bfloat16, tag="sg")
                        nc.gpsimd.tensor_tensor(
                            out=s_t[:], in0=c_t[:], in1=mskb_sb[:],
                            op=mybir.AluOpType.mult)
                        nc.vector.tensor_reduce(
                            out=o_slc,
                            in_=s_t[:].rearrange(
                                "q (G p b) -> q G b p",
                                G=GRPS_PER_TILE, p=GRP, b=B),
                            axis=mybir.AxisListType.X,
                            op=mybir.AluOpType.add)
                    else:
                        if extract == "act":
                            c_t = sp.tile([128, GRPS_PER_TILE * GRP * B],
                                          mybir.dt.bfloat16, tag="ct")
                            nc.scalar.copy(out=c_t[:], in_=ps_t[:])
                            mul_in, s_dt = c_t, mybir.dt.bfloat16
                        else:
                            mul_in, s_dt = ps_t, mybir.dt.float32
                        s_t = sp.tile([128, GRPS_PER_TILE * GRP * B], s_dt,
                                      tag="st")
                        nc.vector.tensor_tensor(
                            out=s_t[:], in0=mul_in[:], in1=msk_sb[:],
                            op=mybir.AluOpType.mult)
                        nc.vector.tensor_reduce(
                            out=o_slc,
                            in_=s_t[:].rearrange(
                                "q (G p b) -> q G b p",
                                G=GRPS_PER_TILE, p=GRP, b=B),
                            axis=mybir.AxisListType.X,
                            op=mybir.AluOpType.add)

            if reps == 1:
                body()
            elif unroll:
                for _ in range(reps):
                    body()
            else:
                with tc.For_i(0, reps, 1) as _i:
                    body(_i)
            nc.sync.dma_start(out=out[:], in_=o_sb[:])
    nc.finalize()
    return nc


def make_mask(extract=None):
    """mask[(pl,k), (G,p',b)] = 1 when p' == pl."""
    extract = EXTRACT if extract is None else extract
    m = np.zeros((GRP, KN, GRPS_PER_TILE, GRP, B), dtype=np.float32)
    for pl in range(GRP):
        m[pl, :, :, pl, :] = 1.0
    m = m.reshape(128, GRPS_PER_TILE * GRP * B)
    if extract == "act":
        import ml_dtypes
        m = m.astype(ml_dtypes.bfloat16)
    return m


def prep_host_inputs(x, weights, hash_idx):
    """Per-core input maps (patches gathered in numpy, all wires bf16).

    m0/m1 layout: [c, (tile, wcols+gcols)] where per tile the first 4096
    cols are weights [p,k] and the next 2048 are patches [p,b]."""
    import ml_dtypes
    B_, Cc, Hh, Ww = x.shape
    flat = np.asarray(x).reshape(B_, -1)
    msk = make_mask()
    in_maps = []
    for c in range(NCORES):
        sl = slice(c * PPC, (c + 1) * PPC)
        w_t = np.asarray(weights)[sl].transpose(2, 0, 1)      # [CK,PPC,KN]
        idx_t = np.asarray(hash_idx)[sl].T                    # [CK,PPC]
        pat = flat[:, idx_t].transpose(1, 2, 0)               # [CK,PPC,B]
        w_t = w_t.reshape(CK, NTILES, WCOL)
        pat = pat.reshape(CK, NTILES, GCOL)
        m = np.concatenate([w_t, pat], axis=2).astype(ml_dtypes.bfloat16)
        m = m.reshape(CK, NTILES * MCOL)
        import ml_dtypes
        in_maps.append({
            "m0": np.ascontiguousarray(m[:C0]),
            "m1": np.ascontiguousarray(m[C0:]),
            "msk": msk,
            "mskb": msk.astype(ml_dtypes.bfloat16),
        })
    return in_maps


def assemble(results, ppc=PPC):
    """Per-core o[(pl,k), (t,G,b)] -> full [B, KN, P]; p = t*256 + G*8 + pl."""
    outs = []
    for r in results:
        o = r["out"].reshape(GRP, KN, ppc // TILE_P, GRPS_PER_TILE, B)
        o = o.transpose(4, 1, 2, 3, 0).reshape(B, KN, ppc)
        outs.append(o)
    return np.concatenate(outs, axis=2)


def kernel(x, weights, hash_idx):
    from concourse.bass_utils import run_bass_kernel_spmd

    if "nc" not in _CACHE:
        _CACHE["nc"] = build()
    nc = _CACHE["nc"]
    in_maps = prep_host_inputs(np.asarray(x), np.asarray(weights),
                               np.asarray(hash_idx))
    res = run_bass_kernel_spmd(nc, in_maps, list(range(NCORES)))
    return assemble(res.results)
